# revision 11
# baseline (speedup 1.0000x reference)
"""Crossformer (cross-attention + MLP block) on 8 Trainium2 NeuronCores.

Sharding:
  - Attention: tensor-parallel over heads (16 heads -> 2 per core). LN weights
    are folded into the projection weights on the host; each core computes
    std(x) / std(ctx) locally (replicated, cheap DVE work), transposes to
    feature-major with the PE, and runs its 2 heads of attention.
  - The output projection is row-split (Megatron); partial sums are combined
    with two bf16 ReduceScatters over row-halves (overlapped with compute).
  - MLP: sequence-sharded. Each core gets 2x128 rows from the ReduceScatters
    and runs the full (LN-folded) MLP on them with resident/streamed weights.
  - Output: core c returns rows [s*1024 + c*128, +128) for s in {0,1}; the
    host reassembles the full [2048, 1024] output.

All matmuls run in bf16 with fp32 PSUM accumulation; LN stats, softmax
denominators, residuals and the final outputs stay fp32.
"""
import math
from contextlib import ExitStack

import numpy as np
import ml_dtypes

import concourse.bass as bass
import concourse.tile as tile
from concourse import bacc, mybir
from concourse.bass_utils import run_bass_kernel_spmd

F32 = mybir.dt.float32
BF16 = mybir.dt.bfloat16
AF = mybir.ActivationFunctionType

NCORES = 8
N, D, C, H, HD, DFF = 2048, 1024, 768, 16, 64, 4096
EPS = 1e-12
HSD = D // NCORES          # feature slice per core (2 heads * 64)
SPLIT = 2                  # number of ReduceScatter pieces
NCH = 512                  # attention query-chunk size
NCHN = N // NCH            # 4 chunks
RPC = N // NCORES          # 256 output rows per core
RPB = RPC // SPLIT         # 128 rows per MLP block
KOX = D // 128             # 8 k-tiles over D
KOC = C // 128             # 6 k-tiles over C
KOF = DFF // 128           # 32 k-tiles over DFF

_cache = {}


def _bcast_ap(ap, parts):
    """Partition-broadcast a [1, F] DRAM AP to [parts, F]."""
    return bass.AP(tensor=ap.tensor, offset=ap.offset,
                   ap=[[0, parts]] + [list(p) for p in ap.ap[1:]])


def build_program():
    nc = bacc.Bacc("TRN2", target_bir_lowering=False, debug=False,
                   num_devices=NCORES)

    def din(name, shape, dt=BF16):
        return nc.dram_tensor(name, shape, dt, kind="ExternalInput").ap()

    x_d = din("x", [N, D], F32)
    ctx_d = din("ctx", [N, C], F32)
    xrows_d = din("xrows", [RPC, D], F32)
    wq_d = din("wq", [D, HSD])
    wk_d = din("wk", [C, HSD])
    wv_d = din("wv", [C, HSD])
    wo0_d = din("wo0", [HD, D])
    wo1_d = din("wo1", [HD, D])
    w1_d = din("w1", [D, DFF])
    w2_d = din("w2", [DFF, D])
    bq_d = din("bq", [HSD, 1], F32)
    bk_d = din("bk", [HSD, 1], F32)
    bv_d = din("bv", [HSD, 1], F32)
    b1_d = din("b1", [1, DFF])
    bo_d = din("bo", [1, D], F32)
    b2_d = din("b2", [1, D], F32)
    id128_d = din("id128", [128, 128])
    id64s_d = din("id64s", [128, HD])
    out_d = nc.dram_tensor("out", [RPC, D], F32, kind="ExternalOutput").ap()

    with tile.TileContext(nc) as tc, ExitStack() as st:
        _build(nc, tc, st, locals())
    nc.compile()
    return nc


def _ln_std(nc, pool, xt, p, d, sub, out_bf, eps_sb):
    """standardize rows of xt [p, d] -> out_bf (bf16). 1-2 bn_stats passes."""
    fmax = math.gcd(512, d)
    nsub = d // fmax
    st = pool.tile([128, nsub, 6], F32, tag="ln_st")
    xg = xt.rearrange("p (s f) -> p s f", s=nsub)
    for g in range(nsub):
        nc.vector.bn_stats(out=st[:p, g, :], in_=xg[:, g, :])
    mv = pool.tile([128, 2], F32, tag="ln_mv")
    nc.vector.bn_aggr(out=mv[:p, :], in_=st[:p])
    sd = pool.tile([128, 1], F32, tag="ln_sd")
    nc.scalar.activation(out=sd[:p], in_=mv[:p, 1:2], func=AF.Sqrt,
                         bias=eps_sb[:p])
    rstd = pool.tile([128, 1], F32, tag="ln_rs")
    nc.vector.reciprocal(out=rstd[:p], in_=sd[:p])
    nc.gpsimd.tensor_scalar(out=out_bf, in0=xt, scalar1=mv[:p, 0:1],
                            scalar2=rstd[:p], op0=mybir.AluOpType.subtract,
                            op1=mybir.AluOpType.mult)
    return rstd


def _build(nc, tc, st, d):
    x_d, ctx_d, xrows_d = d["x_d"], d["ctx_d"], d["xrows_d"]
    wq_d, wk_d, wv_d = d["wq_d"], d["wk_d"], d["wv_d"]
    wo0_d, wo1_d = d["wo0_d"], d["wo1_d"]
    w1_d, w2_d = d["w1_d"], d["w2_d"]
    bq_d, bk_d, bv_d = d["bq_d"], d["bk_d"], d["bv_d"]
    b1_d, bo_d, b2_d = d["b1_d"], d["bo_d"], d["b2_d"]
    id128_d, id64s_d = d["id128_d"], d["id64s_d"]
    out_d = d["out_d"]
    ADD = mybir.AluOpType.add

    const = st.enter_context(tc.tile_pool(name="const", bufs=1))
    persist = st.enter_context(tc.tile_pool(name="persist", bufs=1))
    dram = st.enter_context(tc.tile_pool(name="dram", bufs=1, space="DRAM"))
    mmps = st.enter_context(tc.tile_pool(name="mmps", bufs=3, space="PSUM"))
    trps = st.enter_context(tc.tile_pool(name="trps", bufs=2, space="PSUM"))
    avps = st.enter_context(tc.tile_pool(name="avps", bufs=2, space="PSUM"))

    # constants / biases
    id128 = const.tile([128, 128], BF16)
    nc.sync.dma_start(out=id128, in_=id128_d)
    id64s = const.tile([128, HD], BF16)
    nc.sync.dma_start(out=id64s, in_=id64s_d)
    eps_sb = const.tile([128, 1], F32)
    nc.vector.memset(eps_sb, EPS)
    ones1 = const.tile([1, 128], BF16)
    nc.vector.memset(ones1, 1.0)
    ones1f = const.tile([1, 128], F32)
    nc.vector.memset(ones1f, 1.0)
    bq_sb = const.tile([HSD, 1], F32)
    nc.sync.dma_start(out=bq_sb, in_=bq_d)
    bk_sb = const.tile([HSD, 1], F32)
    nc.sync.dma_start(out=bk_sb, in_=bk_d)
    bv_sb = const.tile([HSD, 1], F32)
    nc.sync.dma_start(out=bv_sb, in_=bv_d)
    b1_sb = const.tile([1, DFF], BF16)
    nc.sync.dma_start(out=b1_sb, in_=b1_d)
    wo0_sb = const.tile([HD, D], BF16)
    nc.sync.dma_start(out=wo0_sb, in_=wo0_d)
    wo1_sb = const.tile([HD, D], BF16)
    nc.sync.dma_start(out=wo1_sb, in_=wo1_d)

    # attention-persistent tensors
    qT = persist.tile([128, N], BF16)
    kT = persist.tile([128, N], BF16)
    vN = persist.tile([128, N // 128, 2 * (HD + 1)], BF16)  # [v_h|1] per head
    oT0 = persist.tile([HD, N], BF16)
    oT1 = persist.tile([HD, N], BF16)
    nc.vector.memset(vN[:, :, HD:HD + 1], 1.0)
    nc.vector.memset(vN[:, :, 2 * HD + 1:], 1.0)

    # DRAM staging for the ReduceScatters
    op_s = [dram.tile([N // SPLIT, D], BF16, name=f"op{s}") for s in range(SPLIT)]
    rs_s = [dram.tile([N // SPLIT // NCORES, D], BF16, name=f"rs{s}")
            for s in range(SPLIT)]

    # ---- phase A: LN + transpose of x and ctx; q/k/v projections ----
    early = tc.tile_pool(name="early", bufs=1)
    with early as ep, tc.tile_pool(name="ebuf", bufs=3) as eb:
        xsT = ep.tile([128, KOX, N], BF16)
        csT = ep.tile([128, KOC, N], BF16)
        vT = ep.tile([128, N], BF16)
        wq_sb = ep.tile([128, KOX, HSD], BF16)
        nc.sync.dma_start(out=wq_sb, in_=wq_d.rearrange("(ko ki) m -> ki ko m", ki=128))
        wk_sb = ep.tile([128, KOC, HSD], BF16)
        nc.sync.dma_start(out=wk_sb, in_=wk_d.rearrange("(ko ki) m -> ki ko m", ki=128))
        wv_sb = ep.tile([128, KOC, HSD], BF16)
        nc.sync.dma_start(out=wv_sb, in_=wv_d.rearrange("(ko ki) m -> ki ko m", ki=128))

        def ln_transpose(src_d, dim, ko, dstT, scope, copy_eng):
            with nc.named_scope(scope):
                for g in range(N // 512):
                    xss = []
                    for t in range(4):
                        i = g * 4 + t
                        xt = eb.tile([128, dim], F32, tag="ln_x")
                        nc.sync.dma_start(out=xt, in_=src_d[i * 128:(i + 1) * 128, :])
                        xs = eb.tile([128, dim], BF16, tag="ln_o", bufs=6)
                        _ln_std(nc, eb, xt, 128, dim, True, xs, eps_sb)
                        xss.append(xs)
                    for j in range(ko):
                        tp = trps.tile([128, 512], BF16, tag="tr")
                        for t in range(4):
                            nc.tensor.transpose(tp[:, t * 128:(t + 1) * 128],
                                                xss[t][:, j * 128:(j + 1) * 128],
                                                id128)
                        copy_eng(out=dstT[:, j, g * 512:(g + 1) * 512], in_=tp)

        ln_transpose(ctx_d, C, KOC, csT, "ln_ctx", nc.scalar.copy)
        ln_transpose(x_d, D, KOX, xsT, "ln_x", nc.vector.tensor_copy)

        with nc.named_scope("proj"):
            for ch in range(NCHN):
                sl = slice(ch * NCH, (ch + 1) * NCH)
                # k
                ps = mmps.tile([128, NCH], F32, tag="mm")
                for k in range(KOC):
                    nc.tensor.matmul(ps, lhsT=wk_sb[:, k, :], rhs=csT[:, k, sl],
                                     start=(k == 0), stop=(k == KOC - 1))
                nc.scalar.activation(out=kT[:, sl], in_=ps, func=AF.Identity,
                                     bias=bk_sb, scale=1.0)
                # v (transposed layout first)
                ps = mmps.tile([128, NCH], F32, tag="mm")
                for k in range(KOC):
                    nc.tensor.matmul(ps, lhsT=wv_sb[:, k, :], rhs=csT[:, k, sl],
                                     start=(k == 0), stop=(k == KOC - 1))
                nc.scalar.activation(out=vT[:, sl], in_=ps, func=AF.Identity,
                                     bias=bv_sb, scale=1.0)
                # q
                ps = mmps.tile([128, NCH], F32, tag="mm")
                for k in range(KOX):
                    nc.tensor.matmul(ps, lhsT=wq_sb[:, k, :], rhs=xsT[:, k, sl],
                                     start=(k == 0), stop=(k == KOX - 1))
                nc.vector.tensor_scalar(out=qT[:, sl], in0=ps, scalar1=bq_sb,
                                        scalar2=None, op0=ADD)

            # v -> row-major with the ones column: vN[:, mo, h*(HD+1) ...]
            for g in range(N // 512):
                for h in range(2):
                    tp = trps.tile([128, 512], BF16, tag="tr")
                    for t in range(4):
                        mo = g * 4 + t
                        nc.tensor.transpose(
                            tp[:, t * 128:t * 128 + HD],
                            vT[h * HD:(h + 1) * HD, mo * 128:(mo + 1) * 128],
                            id64s[h * HD:(h + 1) * HD, :])
                    nc.vector.tensor_copy(
                        out=vN[:, g * 4:(g + 1) * 4, h * (HD + 1):h * (HD + 1) + HD],
                        in_=tp.rearrange("p (t f) -> p t f", t=4)[:, :, :HD])

    # ---- phase B: attention (w1 streams into freed space meanwhile) ----
    wpool = tc.tile_pool(name="wpool", bufs=1)
    with wpool as wp:
        w1_sb = wp.tile([128, KOX, DFF], BF16)
        nc.sync.dma_start(out=w1_sb, in_=w1_d.rearrange("(ko ki) m -> ki ko m", ki=128))
        w2_sb = wp.tile([128, KOF, D], BF16)
        nc.sync.dma_start(out=w2_sb,
                          in_=w2_d.rearrange("(ko ki) m -> ki ko m", ki=128))

        with tc.tile_pool(name="ptp", bufs=2) as ptp, \
             tc.tile_pool(name="opb", bufs=2) as opb, \
             tc.tile_pool(name="att", bufs=2) as att:
            with nc.named_scope("attn"):
                for ch in range(NCHN):
                    sl = slice(ch * NCH, (ch + 1) * NCH)
                    pts = []
                    for mo in range(N // 128):
                        sps = []
                        for h in range(2):
                            hp = slice(h * HD, (h + 1) * HD)
                            if mo == 0:
                                pts.append(ptp.tile([128, N // 128, NCH], BF16,
                                                    tag="pt", name=f"pt{h}"))
                            sp = mmps.tile([128, NCH], F32, tag="mm")
                            nc.tensor.matmul(
                                sp, lhsT=kT[hp, mo * 128:(mo + 1) * 128],
                                rhs=qT[hp, sl], start=True, stop=True)
                            sps.append(sp)
                        for h in range(2):
                            nc.scalar.activation(out=pts[h][:, mo, :], in_=sps[h],
                                                 func=AF.Exp)
                    for h in range(2):
                        pt = pts[h]
                        po = avps.tile([HD + 1, NCH], F32, tag="av")
                        for mo in range(N // 128):
                            nc.tensor.matmul(
                                po, lhsT=vN[:, mo, h * (HD + 1):(h + 1) * (HD + 1)],
                                rhs=pt[:, mo, :], start=(mo == 0),
                                stop=(mo == N // 128 - 1))
                        rsum = att.tile([1, NCH], F32, tag="rec")
                        nc.scalar.copy(out=rsum, in_=po[HD:HD + 1, :])
                        rb_ps = mmps.tile([HD, NCH], F32, tag="mm")
                        nc.tensor.matmul(rb_ps, lhsT=ones1f[:, :HD], rhs=rsum,
                                         start=True, stop=True)
                        rb = att.tile([HD, NCH], F32, tag="rb_sb")
                        nc.vector.reciprocal(out=rb, in_=rb_ps)
                        oT = oT0 if h == 0 else oT1
                        nc.vector.tensor_mul(out=oT[:, sl], in0=po[:HD, :], in1=rb)

                    # o_partial for this chunk of tokens -> DRAM for RS
                    s = ch // (NCHN // SPLIT)
                    base = (ch % (NCHN // SPLIT)) * NCH
                    for nt in range(NCH // 128):
                        osl = slice(ch * NCH + nt * 128, ch * NCH + (nt + 1) * 128)
                        op_t = opb.tile([128, D], BF16, tag="op")
                        for c2 in range(D // 512):
                            pp = mmps.tile([128, 512], F32, tag="mm")
                            nc.tensor.matmul(pp, lhsT=oT0[:, osl],
                                             rhs=wo0_sb[:, c2 * 512:(c2 + 1) * 512],
                                             start=True, stop=False)
                            nc.tensor.matmul(pp, lhsT=oT1[:, osl],
                                             rhs=wo1_sb[:, c2 * 512:(c2 + 1) * 512],
                                             start=False, stop=True)
                            if c2 == 0:
                                nc.vector.tensor_copy(
                                    out=op_t[:, c2 * 512:(c2 + 1) * 512], in_=pp)
                            else:
                                nc.scalar.copy(
                                    out=op_t[:, c2 * 512:(c2 + 1) * 512], in_=pp)
                        nc.sync.dma_start(
                            out=op_s[s][base + nt * 128:base + (nt + 1) * 128, :],
                            in_=op_t)
                    if ch % (NCHN // SPLIT) == NCHN // SPLIT - 1:
                        with nc.named_scope(f"rs{s}"):
                            nc.gpsimd.collective_compute(
                                "ReduceScatter", ADD,
                                replica_groups=[list(range(NCORES))],
                                ins=[op_s[s].opt()], outs=[rs_s[s].opt()])

        # ---- phase C: MLP on this core's rows (2 blocks) ----
        with tc.tile_pool(name="mbuf", bufs=1) as mb:
            bo_b = mb.tile([128, D], F32)
            nc.sync.dma_start(out=bo_b, in_=_bcast_ap(bo_d, 128))
            b2_b = mb.tile([128, D], F32)
            nc.sync.dma_start(out=b2_b, in_=_bcast_ap(b2_d, 128))
            with nc.named_scope("mlp"):
                for s in range(SPLIT):
                    rs_bf = mb.tile([128, D], BF16, tag="rsb")
                    nc.sync.dma_start(out=rs_bf, in_=rs_s[s])
                    xr = mb.tile([128, D], F32, tag="xr")
                    nc.sync.dma_start(out=xr,
                                      in_=xrows_d[s * RPB:(s + 1) * RPB, :])
                    xnew = mb.tile([128, D], F32, tag="xnew")
                    nc.vector.tensor_add(out=xnew, in0=xr, in1=bo_b)
                    nc.vector.tensor_add(out=xnew, in0=xnew, in1=rs_bf)
                    xms = mb.tile([128, D], BF16, tag="xms")
                    _ln_std(nc, mb, xnew, 128, D, True, xms, eps_sb)
                    xmsT = mb.tile([128, KOX, 128], BF16, tag="xmsT")
                    for jg in range(KOX // 4):
                        tp = trps.tile([128, 512], BF16, tag="tr")
                        for t in range(4):
                            j = jg * 4 + t
                            nc.tensor.transpose(tp[:, t * 128:(t + 1) * 128],
                                                xms[:, j * 128:(j + 1) * 128], id128)
                        nc.vector.tensor_copy(
                            out=xmsT[:, jg * 4:(jg + 1) * 4, :],
                            in_=tp.rearrange("p (t f) -> p t f", t=4))
                    # mm1 (N-layout) + bias + gelu
                    g_sb = mb.tile([128, DFF], BF16, tag="g")
                    for ch in range(DFF // 512):
                        pg = mmps.tile([128, 512], F32, tag="mm")
                        for k in range(KOX):
                            nc.tensor.matmul(pg, lhsT=xmsT[:, k, :],
                                             rhs=w1_sb[:, k, ch * 512:(ch + 1) * 512],
                                             start=(k == 0), stop=False)
                        nc.tensor.matmul(pg, lhsT=ones1[:, :128],
                                         rhs=b1_sb[:, ch * 512:(ch + 1) * 512],
                                         start=False, stop=True)
                        nc.scalar.activation(out=g_sb[:, ch * 512:(ch + 1) * 512],
                                             in_=pg, func=AF.Gelu_apprx_tanh)
                    # transpose g
                    gT = mb.tile([128, KOF, 128], BF16, tag="gT")
                    for jg in range(KOF // 4):
                        tp = trps.tile([128, 512], BF16, tag="tr")
                        for t in range(4):
                            j = jg * 4 + t
                            nc.tensor.transpose(tp[:, t * 128:(t + 1) * 128],
                                                g_sb[:, j * 128:(j + 1) * 128], id128)
                        nc.vector.tensor_copy(out=gT[:, jg * 4:(jg + 1) * 4, :],
                                        in_=tp.rearrange("p (t f) -> p t f", t=4))
                    # mm2 + residual + b2
                    out_sb = mb.tile([128, D], F32, tag="osb")
                    for ch in range(D // 512):
                        p2 = mmps.tile([128, 512], F32, tag="mm")
                        for k in range(KOF):
                            nc.tensor.matmul(p2, lhsT=gT[:, k, :],
                                             rhs=w2_sb[:, k, ch * 512:(ch + 1) * 512],
                                             start=(k == 0), stop=(k == KOF - 1))
                        csl = slice(ch * 512, (ch + 1) * 512)
                        nc.vector.tensor_add(out=out_sb[:, csl], in0=p2,
                                             in1=xnew[:, csl])
                        nc.vector.tensor_add(out=out_sb[:, csl], in0=out_sb[:, csl],
                                             in1=b2_b[:, csl])
                    nc.sync.dma_start(out=out_d[s * RPB:(s + 1) * RPB, :],
                                      in_=out_sb)


def shard_inputs(inputs):
    bf = ml_dtypes.bfloat16
    x = np.asarray(inputs["x"], np.float32)
    ctx = np.asarray(inputs["ctx"], np.float32)
    qn_w, qn_b = np.asarray(inputs["qn_w"], np.float32), np.asarray(inputs["qn_b"], np.float32)
    kvn_w, kvn_b = np.asarray(inputs["kvn_w"], np.float32), np.asarray(inputs["kvn_b"], np.float32)
    pn_w, pn_b = np.asarray(inputs["pn_w"], np.float32), np.asarray(inputs["pn_b"], np.float32)
    wq, bq = np.asarray(inputs["wq"], np.float32), np.asarray(inputs["bq"], np.float32)
    wk, bk = np.asarray(inputs["wk"], np.float32), np.asarray(inputs["bk"], np.float32)
    wv, bv = np.asarray(inputs["wv"], np.float32), np.asarray(inputs["bv"], np.float32)
    wo, bo = np.asarray(inputs["wo"], np.float32), np.asarray(inputs["bo"], np.float32)
    w1, b1 = np.asarray(inputs["w1"], np.float32), np.asarray(inputs["b1"], np.float32)
    w2, b2 = np.asarray(inputs["w2"], np.float32), np.asarray(inputs["b2"], np.float32)

    s = 1.0 / math.sqrt(HD)
    wq_f = (qn_w[:, None] * wq) * s
    bq_f = (qn_b @ wq + bq) * s
    wk_f = kvn_w[:, None] * wk
    bk_f = kvn_b @ wk + bk
    wv_f = kvn_w[:, None] * wv
    bv_f = kvn_b @ wv + bv
    w1_f = (pn_w[:, None] * w1).astype(bf)
    b1_f = (pn_b @ w1 + b1).astype(bf).reshape(1, DFF)
    w2_b = w2.astype(bf)
    id128 = np.eye(128, dtype=bf)
    id64s = np.concatenate([np.eye(HD), np.eye(HD)]).astype(bf)

    in_maps = []
    for c in range(NCORES):
        hs = slice(c * HSD, (c + 1) * HSD)
        xrows = np.concatenate(
            [x[sp * (N // SPLIT) + c * RPB: sp * (N // SPLIT) + (c + 1) * RPB]
             for sp in range(SPLIT)])
        woc = wo[hs, :]
        in_maps.append({
            "x": x, "ctx": ctx, "xrows": np.ascontiguousarray(xrows),
            "wq": wq_f[:, hs].astype(bf), "wk": wk_f[:, hs].astype(bf),
            "wv": wv_f[:, hs].astype(bf),
            "wo0": np.ascontiguousarray(woc[:HD]).astype(bf),
            "wo1": np.ascontiguousarray(woc[HD:]).astype(bf),
            "w1": w1_f, "w2": w2_b,
            "bq": bq_f[hs].reshape(-1, 1).astype(np.float32),
            "bk": bk_f[hs].reshape(-1, 1).astype(np.float32),
            "bv": bv_f[hs].reshape(-1, 1).astype(np.float32),
            "b1": b1_f, "bo": bo.reshape(1, -1).astype(np.float32),
            "b2": b2.reshape(1, -1).astype(np.float32),
            "id128": id128, "id64s": id64s,
        })
    return in_maps


def gather_output(results):
    out = np.empty((N, D), np.float32)
    for c in range(NCORES):
        r = results[c]["out"]
        for sp in range(SPLIT):
            out[sp * (N // SPLIT) + c * RPB: sp * (N // SPLIT) + (c + 1) * RPB] = \
                r[sp * RPB:(sp + 1) * RPB]
    return out


def run(inputs, trace=False, **kw):
    if "nc" not in _cache:
        _cache["nc"] = build_program()
    nc = _cache["nc"]
    in_maps = shard_inputs(inputs)
    res = run_bass_kernel_spmd(nc, in_maps, core_ids=list(range(NCORES)),
                               trace=trace, **kw)
    return gather_output(res.results), res


def kernel(**inputs):
    out, _ = run(inputs, trace=False)
    return out


# revision 12
# speedup vs baseline: 1.7987x; 1.7987x over previous
"""Crossformer (cross-attention + MLP block) on 8 Trainium2 NeuronCores.

Sharding:
  - Attention: tensor-parallel over heads (16 heads -> 2 per core). LN weights
    are folded into the projection weights on the host; each core computes
    std(x) / std(ctx) locally (replicated, cheap DVE work), transposes to
    feature-major with the PE, and runs its 2 heads of attention.
  - The output projection is row-split (Megatron); partial sums are combined
    with two bf16 ReduceScatters over row-halves (overlapped with compute).
  - MLP: sequence-sharded. Each core gets 2x128 rows from the ReduceScatters
    and runs the full (LN-folded) MLP on them with resident/streamed weights.
  - Output: core c returns rows [s*1024 + c*128, +128) for s in {0,1}; the
    host reassembles the full [2048, 1024] output.

All matmuls run in bf16 with fp32 PSUM accumulation; LN stats, softmax
denominators, residuals and the final outputs stay fp32.
"""
import math
from contextlib import ExitStack

import numpy as np
import ml_dtypes

import concourse.bass as bass
import concourse.tile as tile
from concourse import bacc, mybir
from concourse.bass_utils import run_bass_kernel_spmd

F32 = mybir.dt.float32
BF16 = mybir.dt.bfloat16
AF = mybir.ActivationFunctionType

NCORES = 8
N, D, C, H, HD, DFF = 2048, 1024, 768, 16, 64, 4096
EPS = 1e-12
HSD = D // NCORES          # feature slice per core (2 heads * 64)
SPLIT = 2                  # number of ReduceScatter pieces
NCH = 512                  # attention query-chunk size
NCHN = N // NCH            # 4 chunks
RPC = N // NCORES          # 256 output rows per core
RPB = RPC // SPLIT         # 128 rows per MLP block
KOX = D // 128             # 8 k-tiles over D
KOC = C // 128             # 6 k-tiles over C
KOF = DFF // 128           # 32 k-tiles over DFF

_cache = {}


def _bcast_ap(ap, parts):
    """Partition-broadcast a [1, F] DRAM AP to [parts, F]."""
    return bass.AP(tensor=ap.tensor, offset=ap.offset,
                   ap=[[0, parts]] + [list(p) for p in ap.ap[1:]])


def build_program():
    nc = bacc.Bacc("TRN2", target_bir_lowering=False, debug=False,
                   num_devices=NCORES)

    def din(name, shape, dt=BF16):
        return nc.dram_tensor(name, shape, dt, kind="ExternalInput").ap()

    x_d = din("x", [N, D], F32)
    ctx_d = din("ctx", [N, C], F32)
    xrows_d = din("xrows", [RPC, D], F32)
    wq_d = din("wq", [D, HSD])
    wk_d = din("wk", [C, HSD])
    wv_d = din("wv", [C, HSD])
    wo0_d = din("wo0", [HD, D])
    wo1_d = din("wo1", [HD, D])
    w1_d = din("w1", [D, DFF])
    w2_d = din("w2", [DFF, D])
    bq_d = din("bq", [HSD, 1], F32)
    bk_d = din("bk", [HSD, 1], F32)
    bv_d = din("bv", [HSD, 1], F32)
    b1_d = din("b1", [1, DFF])
    bo_d = din("bo", [1, D], F32)
    b2_d = din("b2", [1, D], F32)
    id128_d = din("id128", [128, 128])
    id64s_d = din("id64s", [128, HD])
    out_d = nc.dram_tensor("out", [RPC, D], F32, kind="ExternalOutput").ap()

    with tile.TileContext(nc) as tc, ExitStack() as st:
        _build(nc, tc, st, locals())
    nc.compile()
    return nc


def _ln_std(nc, pool, xt, p, d, sub, out_bf, eps_sb):
    """standardize rows of xt [p, d] -> out_bf (bf16). 1-2 bn_stats passes."""
    fmax = math.gcd(512, d)
    nsub = d // fmax
    st = pool.tile([128, nsub, 6], F32, tag="ln_st")
    xg = xt.rearrange("p (s f) -> p s f", s=nsub)
    for g in range(nsub):
        nc.vector.bn_stats(out=st[:p, g, :], in_=xg[:, g, :])
    mv = pool.tile([128, 2], F32, tag="ln_mv")
    nc.vector.bn_aggr(out=mv[:p, :], in_=st[:p])
    sd = pool.tile([128, 1], F32, tag="ln_sd")
    nc.scalar.activation(out=sd[:p], in_=mv[:p, 1:2], func=AF.Sqrt,
                         bias=eps_sb[:p])
    rstd = pool.tile([128, 1], F32, tag="ln_rs")
    nc.vector.reciprocal(out=rstd[:p], in_=sd[:p])
    nc.vector.tensor_scalar(out=out_bf, in0=xt, scalar1=mv[:p, 0:1],
                            scalar2=rstd[:p], op0=mybir.AluOpType.subtract,
                            op1=mybir.AluOpType.mult)
    return rstd


def _build(nc, tc, st, d):
    x_d, ctx_d, xrows_d = d["x_d"], d["ctx_d"], d["xrows_d"]
    wq_d, wk_d, wv_d = d["wq_d"], d["wk_d"], d["wv_d"]
    wo0_d, wo1_d = d["wo0_d"], d["wo1_d"]
    w1_d, w2_d = d["w1_d"], d["w2_d"]
    bq_d, bk_d, bv_d = d["bq_d"], d["bk_d"], d["bv_d"]
    b1_d, bo_d, b2_d = d["b1_d"], d["bo_d"], d["b2_d"]
    id128_d, id64s_d = d["id128_d"], d["id64s_d"]
    out_d = d["out_d"]
    ADD = mybir.AluOpType.add

    const = st.enter_context(tc.tile_pool(name="const", bufs=1))
    persist = st.enter_context(tc.tile_pool(name="persist", bufs=1))
    dram = st.enter_context(tc.tile_pool(name="dram", bufs=1, space="DRAM"))
    mmps = st.enter_context(tc.tile_pool(name="mmps", bufs=3, space="PSUM"))
    trps = st.enter_context(tc.tile_pool(name="trps", bufs=2, space="PSUM"))
    avps = st.enter_context(tc.tile_pool(name="avps", bufs=2, space="PSUM"))

    # constants / biases
    id128 = const.tile([128, 128], BF16)
    nc.sync.dma_start(out=id128, in_=id128_d)
    id64s = const.tile([128, HD], BF16)
    nc.sync.dma_start(out=id64s, in_=id64s_d)
    eps_sb = const.tile([128, 1], F32)
    nc.vector.memset(eps_sb, EPS)
    ones1 = const.tile([1, 128], BF16)
    nc.vector.memset(ones1, 1.0)
    ones1f = const.tile([1, 128], F32)
    nc.vector.memset(ones1f, 1.0)
    bq_sb = const.tile([HSD, 1], F32)
    nc.sync.dma_start(out=bq_sb, in_=bq_d)
    bk_sb = const.tile([HSD, 1], F32)
    nc.sync.dma_start(out=bk_sb, in_=bk_d)
    bv_sb = const.tile([HSD, 1], F32)
    nc.sync.dma_start(out=bv_sb, in_=bv_d)
    b1_sb = const.tile([1, DFF], BF16)
    nc.sync.dma_start(out=b1_sb, in_=b1_d)
    wo0_sb = const.tile([HD, D], BF16)
    nc.sync.dma_start(out=wo0_sb, in_=wo0_d)
    wo1_sb = const.tile([HD, D], BF16)
    nc.sync.dma_start(out=wo1_sb, in_=wo1_d)

    # attention-persistent tensors
    qT = persist.tile([128, N], BF16)
    kT = persist.tile([128, N], BF16)
    vN = persist.tile([128, N // 128, 2 * (HD + 1)], BF16)  # [v_h|1] per head
    oT0 = persist.tile([HD, N], BF16)
    oT1 = persist.tile([HD, N], BF16)
    nc.vector.memset(vN[:, :, HD:HD + 1], 1.0)
    nc.vector.memset(vN[:, :, 2 * HD + 1:], 1.0)

    # DRAM staging for the ReduceScatters
    op_s = [dram.tile([N // SPLIT, D], BF16, name=f"op{s}") for s in range(SPLIT)]
    rs_s = [dram.tile([N // SPLIT // NCORES, D], BF16, name=f"rs{s}")
            for s in range(SPLIT)]

    # ---- phase A: LN + transpose of x and ctx; q/k/v projections ----
    early = tc.tile_pool(name="early", bufs=1)
    with early as ep, tc.tile_pool(name="ebuf", bufs=3) as eb:
        xsT = ep.tile([128, KOX, N], BF16)
        csT = ep.tile([128, KOC, N], BF16)
        vT = ep.tile([128, N], BF16)
        wq_sb = ep.tile([128, KOX, HSD], BF16)
        nc.sync.dma_start(out=wq_sb, in_=wq_d.rearrange("(ko ki) m -> ki ko m", ki=128))
        wk_sb = ep.tile([128, KOC, HSD], BF16)
        nc.sync.dma_start(out=wk_sb, in_=wk_d.rearrange("(ko ki) m -> ki ko m", ki=128))
        wv_sb = ep.tile([128, KOC, HSD], BF16)
        nc.sync.dma_start(out=wv_sb, in_=wv_d.rearrange("(ko ki) m -> ki ko m", ki=128))

        def ln_transpose(src_d, dim, ko, dstT, scope, copy_eng):
            with nc.named_scope(scope):
                for g in range(N // 512):
                    xss = []
                    for t in range(4):
                        i = g * 4 + t
                        xt = eb.tile([128, dim], F32, tag="ln_x")
                        nc.sync.dma_start(out=xt, in_=src_d[i * 128:(i + 1) * 128, :])
                        xs = eb.tile([128, dim], BF16, tag="ln_o", bufs=6)
                        _ln_std(nc, eb, xt, 128, dim, True, xs, eps_sb)
                        xss.append(xs)
                    for j in range(ko):
                        tp = trps.tile([128, 512], BF16, tag="tr")
                        for t in range(4):
                            nc.tensor.transpose(tp[:, t * 128:(t + 1) * 128],
                                                xss[t][:, j * 128:(j + 1) * 128],
                                                id128)
                        copy_eng(out=dstT[:, j, g * 512:(g + 1) * 512], in_=tp)

        ln_transpose(ctx_d, C, KOC, csT, "ln_ctx", nc.scalar.copy)
        ln_transpose(x_d, D, KOX, xsT, "ln_x", nc.vector.tensor_copy)

        with nc.named_scope("proj"):
            for ch in range(NCHN):
                sl = slice(ch * NCH, (ch + 1) * NCH)
                # k
                ps = mmps.tile([128, NCH], F32, tag="mm")
                for k in range(KOC):
                    nc.tensor.matmul(ps, lhsT=wk_sb[:, k, :], rhs=csT[:, k, sl],
                                     start=(k == 0), stop=(k == KOC - 1))
                nc.scalar.activation(out=kT[:, sl], in_=ps, func=AF.Identity,
                                     bias=bk_sb, scale=1.0)
                # v (transposed layout first)
                ps = mmps.tile([128, NCH], F32, tag="mm")
                for k in range(KOC):
                    nc.tensor.matmul(ps, lhsT=wv_sb[:, k, :], rhs=csT[:, k, sl],
                                     start=(k == 0), stop=(k == KOC - 1))
                nc.scalar.activation(out=vT[:, sl], in_=ps, func=AF.Identity,
                                     bias=bv_sb, scale=1.0)
                # q
                ps = mmps.tile([128, NCH], F32, tag="mm")
                for k in range(KOX):
                    nc.tensor.matmul(ps, lhsT=wq_sb[:, k, :], rhs=xsT[:, k, sl],
                                     start=(k == 0), stop=(k == KOX - 1))
                nc.vector.tensor_scalar(out=qT[:, sl], in0=ps, scalar1=bq_sb,
                                        scalar2=None, op0=ADD)

            # v -> row-major with the ones column: vN[:, mo, h*(HD+1) ...]
            for g in range(N // 512):
                for h in range(2):
                    tp = trps.tile([128, 512], BF16, tag="tr")
                    for t in range(4):
                        mo = g * 4 + t
                        nc.tensor.transpose(
                            tp[:, t * 128:t * 128 + HD],
                            vT[h * HD:(h + 1) * HD, mo * 128:(mo + 1) * 128],
                            id64s[h * HD:(h + 1) * HD, :])
                    nc.vector.tensor_copy(
                        out=vN[:, g * 4:(g + 1) * 4, h * (HD + 1):h * (HD + 1) + HD],
                        in_=tp.rearrange("p (t f) -> p t f", t=4)[:, :, :HD])

    # ---- phase B: attention (w1 streams into freed space meanwhile) ----
    wpool = tc.tile_pool(name="wpool", bufs=1)
    with wpool as wp:
        w1_sb = wp.tile([128, KOX, DFF], BF16)
        nc.sync.dma_start(out=w1_sb, in_=w1_d.rearrange("(ko ki) m -> ki ko m", ki=128))
        w2_sb = wp.tile([128, KOF, D], BF16)
        nc.sync.dma_start(out=w2_sb,
                          in_=w2_d.rearrange("(ko ki) m -> ki ko m", ki=128))

        with tc.tile_pool(name="ptp", bufs=2) as ptp, \
             tc.tile_pool(name="opb", bufs=2) as opb, \
             tc.tile_pool(name="att", bufs=2) as att:
            with nc.named_scope("attn"):
                for ch in range(NCHN):
                    sl = slice(ch * NCH, (ch + 1) * NCH)
                    pts = []
                    for mo in range(N // 128):
                        sps = []
                        for h in range(2):
                            hp = slice(h * HD, (h + 1) * HD)
                            if mo == 0:
                                pts.append(ptp.tile([128, N // 128, NCH], BF16,
                                                    tag="pt", name=f"pt{h}"))
                            sp = mmps.tile([128, NCH], F32, tag="mm")
                            nc.tensor.matmul(
                                sp, lhsT=kT[hp, mo * 128:(mo + 1) * 128],
                                rhs=qT[hp, sl], start=True, stop=True)
                            sps.append(sp)
                        for h in range(2):
                            nc.scalar.activation(out=pts[h][:, mo, :], in_=sps[h],
                                                 func=AF.Exp)
                    for h in range(2):
                        pt = pts[h]
                        po = avps.tile([HD + 1, NCH], F32, tag="av")
                        for mo in range(N // 128):
                            nc.tensor.matmul(
                                po, lhsT=vN[:, mo, h * (HD + 1):(h + 1) * (HD + 1)],
                                rhs=pt[:, mo, :], start=(mo == 0),
                                stop=(mo == N // 128 - 1))
                        rsum = att.tile([1, NCH], F32, tag="rec")
                        nc.scalar.copy(out=rsum, in_=po[HD:HD + 1, :])
                        rb_ps = mmps.tile([HD, NCH], F32, tag="mm")
                        nc.tensor.matmul(rb_ps, lhsT=ones1f[:, :HD], rhs=rsum,
                                         start=True, stop=True)
                        rb = att.tile([HD, NCH], F32, tag="rb_sb")
                        nc.vector.reciprocal(out=rb, in_=rb_ps)
                        oT = oT0 if h == 0 else oT1
                        nc.vector.tensor_mul(out=oT[:, sl], in0=po[:HD, :], in1=rb)

                    # o_partial for this chunk of tokens -> DRAM for RS
                    s = ch // (NCHN // SPLIT)
                    base = (ch % (NCHN // SPLIT)) * NCH
                    for nt in range(NCH // 128):
                        osl = slice(ch * NCH + nt * 128, ch * NCH + (nt + 1) * 128)
                        op_t = opb.tile([128, D], BF16, tag="op")
                        for c2 in range(D // 512):
                            pp = mmps.tile([128, 512], F32, tag="mm")
                            nc.tensor.matmul(pp, lhsT=oT0[:, osl],
                                             rhs=wo0_sb[:, c2 * 512:(c2 + 1) * 512],
                                             start=True, stop=False)
                            nc.tensor.matmul(pp, lhsT=oT1[:, osl],
                                             rhs=wo1_sb[:, c2 * 512:(c2 + 1) * 512],
                                             start=False, stop=True)
                            if c2 == 0:
                                nc.vector.tensor_copy(
                                    out=op_t[:, c2 * 512:(c2 + 1) * 512], in_=pp)
                            else:
                                nc.scalar.copy(
                                    out=op_t[:, c2 * 512:(c2 + 1) * 512], in_=pp)
                        nc.sync.dma_start(
                            out=op_s[s][base + nt * 128:base + (nt + 1) * 128, :],
                            in_=op_t)
                    if ch % (NCHN // SPLIT) == NCHN // SPLIT - 1:
                        with nc.named_scope(f"rs{s}"):
                            nc.gpsimd.collective_compute(
                                "ReduceScatter", ADD,
                                replica_groups=[list(range(NCORES))],
                                ins=[op_s[s].opt()], outs=[rs_s[s].opt()])

        # ---- phase C: MLP on this core's rows (2 blocks) ----
        with tc.tile_pool(name="mbuf", bufs=1) as mb:
            bo_b = mb.tile([128, D], F32)
            nc.sync.dma_start(out=bo_b, in_=_bcast_ap(bo_d, 128))
            b2_b = mb.tile([128, D], F32)
            nc.sync.dma_start(out=b2_b, in_=_bcast_ap(b2_d, 128))
            with nc.named_scope("mlp"):
                for s in range(SPLIT):
                    rs_bf = mb.tile([128, D], BF16, tag="rsb")
                    nc.sync.dma_start(out=rs_bf, in_=rs_s[s])
                    xr = mb.tile([128, D], F32, tag="xr")
                    nc.sync.dma_start(out=xr,
                                      in_=xrows_d[s * RPB:(s + 1) * RPB, :])
                    xnew = mb.tile([128, D], F32, tag="xnew")
                    nc.vector.tensor_add(out=xnew, in0=xr, in1=bo_b)
                    nc.vector.tensor_add(out=xnew, in0=xnew, in1=rs_bf)
                    xms = mb.tile([128, D], BF16, tag="xms")
                    _ln_std(nc, mb, xnew, 128, D, True, xms, eps_sb)
                    xmsT = mb.tile([128, KOX, 128], BF16, tag="xmsT")
                    for jg in range(KOX // 4):
                        tp = trps.tile([128, 512], BF16, tag="tr")
                        for t in range(4):
                            j = jg * 4 + t
                            nc.tensor.transpose(tp[:, t * 128:(t + 1) * 128],
                                                xms[:, j * 128:(j + 1) * 128], id128)
                        nc.vector.tensor_copy(
                            out=xmsT[:, jg * 4:(jg + 1) * 4, :],
                            in_=tp.rearrange("p (t f) -> p t f", t=4))
                    # mm1 (N-layout) + bias + gelu
                    g_sb = mb.tile([128, DFF], BF16, tag="g")
                    for ch in range(DFF // 512):
                        pg = mmps.tile([128, 512], F32, tag="mm")
                        for k in range(KOX):
                            nc.tensor.matmul(pg, lhsT=xmsT[:, k, :],
                                             rhs=w1_sb[:, k, ch * 512:(ch + 1) * 512],
                                             start=(k == 0), stop=False)
                        nc.tensor.matmul(pg, lhsT=ones1[:, :128],
                                         rhs=b1_sb[:, ch * 512:(ch + 1) * 512],
                                         start=False, stop=True)
                        nc.scalar.activation(out=g_sb[:, ch * 512:(ch + 1) * 512],
                                             in_=pg, func=AF.Gelu_apprx_tanh)
                    # transpose g
                    gT = mb.tile([128, KOF, 128], BF16, tag="gT")
                    for jg in range(KOF // 4):
                        tp = trps.tile([128, 512], BF16, tag="tr")
                        for t in range(4):
                            j = jg * 4 + t
                            nc.tensor.transpose(tp[:, t * 128:(t + 1) * 128],
                                                g_sb[:, j * 128:(j + 1) * 128], id128)
                        nc.vector.tensor_copy(out=gT[:, jg * 4:(jg + 1) * 4, :],
                                        in_=tp.rearrange("p (t f) -> p t f", t=4))
                    # mm2 + residual + b2
                    out_sb = mb.tile([128, D], F32, tag="osb")
                    for ch in range(D // 512):
                        p2 = mmps.tile([128, 512], F32, tag="mm")
                        for k in range(KOF):
                            nc.tensor.matmul(p2, lhsT=gT[:, k, :],
                                             rhs=w2_sb[:, k, ch * 512:(ch + 1) * 512],
                                             start=(k == 0), stop=(k == KOF - 1))
                        csl = slice(ch * 512, (ch + 1) * 512)
                        nc.vector.tensor_add(out=out_sb[:, csl], in0=p2,
                                             in1=xnew[:, csl])
                        nc.vector.tensor_add(out=out_sb[:, csl], in0=out_sb[:, csl],
                                             in1=b2_b[:, csl])
                    nc.sync.dma_start(out=out_d[s * RPB:(s + 1) * RPB, :],
                                      in_=out_sb)


def shard_inputs(inputs):
    bf = ml_dtypes.bfloat16
    x = np.asarray(inputs["x"], np.float32)
    ctx = np.asarray(inputs["ctx"], np.float32)
    qn_w, qn_b = np.asarray(inputs["qn_w"], np.float32), np.asarray(inputs["qn_b"], np.float32)
    kvn_w, kvn_b = np.asarray(inputs["kvn_w"], np.float32), np.asarray(inputs["kvn_b"], np.float32)
    pn_w, pn_b = np.asarray(inputs["pn_w"], np.float32), np.asarray(inputs["pn_b"], np.float32)
    wq, bq = np.asarray(inputs["wq"], np.float32), np.asarray(inputs["bq"], np.float32)
    wk, bk = np.asarray(inputs["wk"], np.float32), np.asarray(inputs["bk"], np.float32)
    wv, bv = np.asarray(inputs["wv"], np.float32), np.asarray(inputs["bv"], np.float32)
    wo, bo = np.asarray(inputs["wo"], np.float32), np.asarray(inputs["bo"], np.float32)
    w1, b1 = np.asarray(inputs["w1"], np.float32), np.asarray(inputs["b1"], np.float32)
    w2, b2 = np.asarray(inputs["w2"], np.float32), np.asarray(inputs["b2"], np.float32)

    s = 1.0 / math.sqrt(HD)
    wq_f = (qn_w[:, None] * wq) * s
    bq_f = (qn_b @ wq + bq) * s
    wk_f = kvn_w[:, None] * wk
    bk_f = kvn_b @ wk + bk
    wv_f = kvn_w[:, None] * wv
    bv_f = kvn_b @ wv + bv
    w1_f = (pn_w[:, None] * w1).astype(bf)
    b1_f = (pn_b @ w1 + b1).astype(bf).reshape(1, DFF)
    w2_b = w2.astype(bf)
    id128 = np.eye(128, dtype=bf)
    id64s = np.concatenate([np.eye(HD), np.eye(HD)]).astype(bf)

    in_maps = []
    for c in range(NCORES):
        hs = slice(c * HSD, (c + 1) * HSD)
        xrows = np.concatenate(
            [x[sp * (N // SPLIT) + c * RPB: sp * (N // SPLIT) + (c + 1) * RPB]
             for sp in range(SPLIT)])
        woc = wo[hs, :]
        in_maps.append({
            "x": x, "ctx": ctx, "xrows": np.ascontiguousarray(xrows),
            "wq": wq_f[:, hs].astype(bf), "wk": wk_f[:, hs].astype(bf),
            "wv": wv_f[:, hs].astype(bf),
            "wo0": np.ascontiguousarray(woc[:HD]).astype(bf),
            "wo1": np.ascontiguousarray(woc[HD:]).astype(bf),
            "w1": w1_f, "w2": w2_b,
            "bq": bq_f[hs].reshape(-1, 1).astype(np.float32),
            "bk": bk_f[hs].reshape(-1, 1).astype(np.float32),
            "bv": bv_f[hs].reshape(-1, 1).astype(np.float32),
            "b1": b1_f, "bo": bo.reshape(1, -1).astype(np.float32),
            "b2": b2.reshape(1, -1).astype(np.float32),
            "id128": id128, "id64s": id64s,
        })
    return in_maps


def gather_output(results):
    out = np.empty((N, D), np.float32)
    for c in range(NCORES):
        r = results[c]["out"]
        for sp in range(SPLIT):
            out[sp * (N // SPLIT) + c * RPB: sp * (N // SPLIT) + (c + 1) * RPB] = \
                r[sp * RPB:(sp + 1) * RPB]
    return out


def run(inputs, trace=False, **kw):
    if "nc" not in _cache:
        _cache["nc"] = build_program()
    nc = _cache["nc"]
    in_maps = shard_inputs(inputs)
    res = run_bass_kernel_spmd(nc, in_maps, core_ids=list(range(NCORES)),
                               trace=trace, **kw)
    return gather_output(res.results), res


def kernel(**inputs):
    out, _ = run(inputs, trace=False)
    return out
